# revision 19
# baseline (speedup 1.0000x reference)
# Trainium2 Bass kernel: Llama-style attention block (GQA + RoPE + causal),
# tensor-parallel across heads on 8 NeuronCores.
#
# Full-shape contract: kernel(**inputs) takes the unsharded numpy inputs and
# returns the full [B, S, HID] float32 output.
#
# Sharding strategy (per core i of 8):
#   - 4 query heads (rows i*512:(i+1)*512 of Wq) + 1 kv head (rows i*128.. of Wk/Wv)
#   - Wo is sharded row-wise (its columns i*512:(i+1)*512); each core emits a
#     partial [B,S,HID] product (bf16) which the host sums at gather time.
# All weights/activations are pre-transposed and pre-tiled on the host so the
# device kernel needs zero on-chip transposes of activations:
#   matmul(out[M,N], lhsT[K,M], rhs[K,N]) contracts over the partition dim K.
#
# Schedule: fully chunk-interleaved so the PE never idles long enough for the
# HAM clock gate to re-throttle, and so the scalar engine's exp work (which
# alone nearly matches the PE's score+AV matmul time) always has PE work to
# hide behind:
#   per 512-token chunk: QKV projection sweep -> RoPE -> V transpose ->
#   attention for the chunk's 4 heads, with (a) score matmuls software-
#   pipelined two steps ahead of the attn*V matmuls, (b) the softmax
#   normalization (all-bf16 sum + broadcast matmuls, fast-approx reciprocal)
#   deferred into the NEXT head's score stream, and (c) the PREVIOUS chunk's
#   output-projection groups interleaved one-per-kt into the score stream so
#   the PE stays busy when the exp stream is the rate limiter.
# Scores stay transposed [k, q]; softmax is unnormalized (scores are O(10) so
# exp is safe in fp32); the normalizer is folded in at the attention-output
# eviction.

import os
import sys
from contextlib import ExitStack

for _p in ("/opt/trn_rl_repo", "/root/.axon_site/_ro/trn_rl_repo"):
    if os.path.isdir(_p) and _p not in sys.path:
        sys.path.append(_p)

import ml_dtypes
import numpy as np

import concourse.bass as bass
import concourse.mybir as mybir
import concourse.tile as tile
from concourse import bacc
from concourse.bass_utils import run_bass_kernel_spmd

BF16 = mybir.dt.bfloat16
F32 = mybir.dt.float32
NEG = -1.0e9
N_CORES = 8


def build_core_kernel(B, S, HID, QH=4, D=128, QCH=512):
    """SPMD per-core program. QH query heads + 1 kv head per core.

    DRAM parameter layouts (host pre-tiles everything):
      ht   [B, KT, TC, 128, QCH]  bf16  hidden^T tiles: [b,kt,tc,i,j] = hidden[b, tc*QCH+j, kt*128+i]
      trig [B, 2, D, S]           bf16  cos^T / sin^T
      wq   [KT, 128, QH*D]        bf16  Wq_core^T tiles (contraction rows blocked by 128)
      wk   [KT, 128, D]           bf16
      wv   [KT, 128, D]           bf16
      wo   [QH, D, HID]           bf16  Wo_core^T rows blocked per head
      mask [128, QCH]             f32   additive causal triangle (cols 0:128), 0 beyond
      ones [128, 1]               bf16  partition-sum lhsT
      onesr [1, 128]              bf16  broadcast lhsT
      eye  [128, 128]             bf16
      out  [B, TT, HC, 128, QCH]  bf16  partial output tiles
    """
    FS = QH * D          # per-core feature slice of the qkv/attn space
    KT = HID // 128      # contraction tiles for projections
    TC = S // QCH        # 512-token chunks
    TT = S // 128        # 128-token tiles
    TPC = QCH // 128     # token tiles per chunk
    HC = HID // QCH      # output hid chunks
    KPQ = QCH // 128     # k-tiles per q-chunk (diagonal band width)
    HALF = D // 2
    PIPE = 2             # score->AV software pipeline depth
    SC = float(1.0 / np.sqrt(D))
    EXP = mybir.ActivationFunctionType.Exp

    # Bacc (not plain Bass): its compile pipeline splits multi-sem waits into
    # EventSemaphore instructions — the DMA DIRECT2D struct has one wait slot.
    nc = bacc.Bacc(None)
    ht = nc.declare_dram_parameter("ht", [B, KT, TC, 128, QCH], BF16, isOutput=False)
    trig = nc.declare_dram_parameter("trig", [B, 2, D, S], BF16, isOutput=False)
    wq = nc.declare_dram_parameter("wq", [KT, 128, FS], BF16, isOutput=False)
    wk = nc.declare_dram_parameter("wk", [KT, 128, D], BF16, isOutput=False)
    wv = nc.declare_dram_parameter("wv", [KT, 128, D], BF16, isOutput=False)
    wo = nc.declare_dram_parameter("wo", [QH, D, HID], BF16, isOutput=False)
    mask = nc.declare_dram_parameter("mask", [128, QCH], F32, isOutput=False)
    ones = nc.declare_dram_parameter("ones", [128, 1], BF16, isOutput=False)
    onesr = nc.declare_dram_parameter("onesr", [1, 128], BF16, isOutput=False)
    eye = nc.declare_dram_parameter("eye", [128, 128], BF16, isOutput=False)
    out = nc.declare_dram_parameter("out", [B, TT, HC, 128, QCH], BF16, isOutput=True)

    with ExitStack() as ctx:
        tc = ctx.enter_context(tile.TileContext(nc))
        pool = lambda name, bufs, space=None: ctx.enter_context(
            tc.tile_pool(name=name, bufs=bufs, **({"space": space} if space else {}))
        )
        p_w = pool("p_w", 1)          # weights + constants, loaded once
        p_ht = pool("p_ht", 35)       # streamed hidden^T tiles (full chunk + prefetch)
        p_qt = pool("p_qt", 8)        # per-chunk per-head Q^T [D, QCH] bf16
        p_kt = pool("p_kt", 1)        # K^T [D, S] bf16 (persistent per batch)
        p_vtt = pool("p_vtt", 3)      # V^T staging before transpose
        p_vt = pool("p_vt", S // 128 + 4)  # V tiles [128 tok, D]
        p_at = pool("p_at", 8)        # per-chunk per-head attn^T [D, QCH] bf16
        p_exp = pool("p_exp", 7)      # exp(score) tiles bf16
        p_rt = pool("p_rt", 1)        # rope temps f32
        p_acc = pool("p_acc", 2)      # softmax-sum accumulators bf16
        p_sums = pool("p_sums", 2)    # softmax sums row bf16
        p_rbc = pool("p_rbc", 2)      # broadcast reciprocal f32
        p_ost = pool("p_ost", 4)      # output staging bf16
        p_pp = pool("p_pp", 5, "PSUM")    # general [128, QCH] psum ring
        p_av = pool("p_av", 2, "PSUM")    # attn-v accumulators (+proj V)
        p_sm = pool("p_sm", 1, "PSUM")    # softmax sum rows

        wq_sb = p_w.tile([128, KT, FS], BF16, name="wq_sb")
        wk_sb = p_w.tile([128, KT, D], BF16, name="wk_sb")
        wv_sb = p_w.tile([128, KT, D], BF16, name="wv_sb")
        wo_sb = p_w.tile([128, QH, HID], BF16, name="wo_sb")
        mask_sb = p_w.tile([128, QCH], F32, name="mask_sb")
        ones_sb = p_w.tile([128, 1], BF16, name="ones_sb")
        onesr_sb = p_w.tile([1, 128], BF16, name="onesr_sb")
        eye_sb = p_w.tile([128, 128], BF16, name="eye_sb")
        cos_sb = p_w.tile([D, B, S], BF16, name="cos_sb")
        sin_sb = p_w.tile([D, B, S], BF16, name="sin_sb")
        # Startup order: tiny constants first, then per-kt weights split
        # across the two DMA queues so the first projection sweep's operands
        # stay ahead of the PE's ~1.1 us/kt consumption rate. cos/sin are
        # needed ~35 us in (first RoPE eviction), wo at ~90 us (first outproj).
        nc.scalar.dma_start(out=mask_sb[:, :], in_=mask[:, :])
        nc.scalar.dma_start(out=ones_sb[:, :], in_=ones[:, :])
        nc.scalar.dma_start(out=onesr_sb[:, :], in_=onesr[:, :])
        nc.scalar.dma_start(out=eye_sb[:, :], in_=eye[:, :])
        for kt in range(KT):
            nc.scalar.dma_start(out=wq_sb[:, kt, :], in_=wq[kt])
            nc.sync.dma_start(out=wk_sb[:, kt, :], in_=wk[kt])
            nc.sync.dma_start(out=wv_sb[:, kt, :], in_=wv[kt])
        for b in range(B):
            nc.scalar.dma_start(out=cos_sb[:, b, :], in_=trig[b, 0])
            nc.scalar.dma_start(out=sin_sb[:, b, :], in_=trig[b, 1])
        for f in range(QH):
            nc.scalar.dma_start(out=wo_sb[:, f, :], in_=wo[f])

        def rope_evict(b, dst, ps, tci):
            # dst[:, :] = ps * cos + rotate_half(ps) * sin  (write bf16, chunk tci)
            # bf16 temporaries: DVE runs 2 elems/cycle for 16-bit vs 1 for f32.
            sl = slice(tci * QCH, (tci + 1) * QCH)
            cs = cos_sb[:, b, sl]
            sn = sin_sb[:, b, sl]
            t1 = p_rt.tile([128, QCH], BF16, name="rt1", tag="rt1")
            t2 = p_rt.tile([128, QCH], BF16, name="rt2", tag="rt2")
            nc.vector.tensor_mul(t1[:, :], ps[:, :], cs)
            nc.vector.tensor_mul(t2[0:HALF, :], ps[HALF:D, :], sn[0:HALF, :])
            nc.vector.tensor_mul(t2[HALF:D, :], ps[0:HALF, :], sn[HALF:D, :])
            nc.vector.tensor_sub(dst[0:HALF, :], t1[0:HALF, :], t2[0:HALF, :])
            nc.vector.tensor_add(dst[HALF:D, :], t1[HALF:D, :], t2[HALF:D, :])

        # Deferred softmax normalization: `pend` carries (acc, av, at_dst) of
        # the previous head; its PE ops (sum + broadcast matmuls, all bf16)
        # are issued inside the NEXT head's score stream so their
        # vector/scalar dependencies are long satisfied.
        def emit_norm_sum(pend):
            acc, av, at_dst = pend
            sm = p_sm.tile([1, QCH], F32, name="ps_sm", tag="sm")
            nc.tensor.matmul(sm[:, :], ones_sb[:, :], acc[:, :],
                             start=True, stop=True)
            sms = p_sums.tile([1, QCH], BF16, name="sms", tag="sums")
            nc.scalar.copy(sms[:, :], sm[:, :])
            return (av, at_dst, sms)

        def emit_norm_apply(pend2):
            av, at_dst, sms = pend2
            # broadcast sums down partitions via K=1 outer product
            pb = p_pp.tile([128, QCH], F32, name="ps_pb", tag="pp")
            nc.tensor.matmul(pb[:, :], onesr_sb[:, :], sms[:, :],
                             start=True, stop=True)
            rb = p_rbc.tile([128, QCH], F32, name="rb", tag="rb")
            nc.vector.reciprocal_approx_fast(rb[:, :], pb[:, :])
            nc.vector.tensor_mul(at_dst[:, :], av[:, :], rb[:, :])

        # Output projection emitters for one chunk (32 (tt,hc) groups); popped
        # one-per-kt into the next chunk's attention score stream.
        def emit_op_group(g):
            bb, tt, tt4, hc, atp = g
            csl = slice(tt4 * 128, (tt4 + 1) * 128)
            pw = p_pp.tile([128, QCH], F32, name="ps_wo", tag="pp")
            for f in range(QH):
                nc.tensor.matmul(pw[:, :], atp[f][:, csl],
                                 wo_sb[:, f, hc * QCH:(hc + 1) * QCH],
                                 start=(f == 0), stop=(f == QH - 1))
            o = p_ost.tile([128, QCH], BF16, name="o_t", tag="ost")
            if hc % 2 == 0:
                nc.vector.tensor_copy(o[:, :], pw[:, :])
            else:
                nc.scalar.copy(o[:, :], pw[:, :])
            nc.sync.dma_start(out=out[bb, tt, hc], in_=o[:, :])

        def make_op_groups(bb, tcip, atp):
            return [(bb, tcip * TPC + tt4, tt4, hc, atp)
                    for tt4 in range(TPC) for hc in range(HC)]

        pend = None       # deferred normalization state (crosses chunks)
        pend2 = None
        prev_op = None    # (b, tci, at_c) of the chunk awaiting outproj

        for b in range(B):
            vb = []       # V tiles [128 tok, D], global k-tile index
            ktb = p_kt.tile([D, S], BF16, name="ktb", tag="kt")
            for tci in range(TC):
                # ---- projection sweep 1: Q heads + K accumulate per kt ----
                qps = [p_pp.tile([128, QCH], F32, name=f"ps_q{f}", tag="pp")
                       for f in range(QH)]
                kps = p_pp.tile([128, QCH], F32, name="ps_k", tag="pp")
                hts = []
                for kt in range(KT):
                    t = p_ht.tile([128, QCH], BF16, name="ht_t", tag="ht")
                    nc.sync.dma_start(out=t[:, :], in_=ht[b, kt, tci])
                    hts.append(t)
                    st_, sp_ = (kt == 0), (kt == KT - 1)
                    for f in range(QH):
                        nc.tensor.matmul(qps[f][:, :],
                                         wq_sb[:, kt, f * D:(f + 1) * D],
                                         t[:, :], start=st_, stop=sp_)
                    nc.tensor.matmul(kps[:, :], wk_sb[:, kt, :], t[:, :],
                                     start=st_, stop=sp_)
                qtb_c = [p_qt.tile([D, QCH], BF16, name=f"qtc{f}", tag="qt")
                         for f in range(QH)]
                ktb_c = ktb[:, tci * QCH:(tci + 1) * QCH]
                # RoPE order on the DVE: K first (the score matmuls' stationary
                # operand), then q0/q1; vtt is copied mid-stream so the PE's
                # transposes unblock right after sweep 2; q2/q3 follow (their
                # heads start much later).
                rope_evict(b, ktb_c, kps, tci)
                for f in range(QH):
                    rope_evict(b, qtb_c[f], qps[f], tci)
                # ---- sweep 2: V^T accumulation, then PE-transpose ----
                oq = make_op_groups(*prev_op) if prev_op else []
                vps = p_av.tile([128, QCH], F32, name="ps_v", tag="av")
                for kt in range(KT):
                    nc.tensor.matmul(vps[:, :], wv_sb[:, kt, :], hts[kt][:, :],
                                     start=(kt == 0), stop=(kt == KT - 1))
                vtt = p_vtt.tile([128, QCH], BF16, name="vtt", tag="vtt")
                nc.vector.tensor_copy(vtt[:, :], vps[:, :])
                # two early outproj pops cover the vtt-copy latency before the
                # transposes can start (safe: at tiles of the previous chunk
                # are fully normalized by that chunk's end)
                for _ in range(2):
                    if oq:
                        emit_op_group(oq.pop(0))
                for sub in range(TPC):
                    pt = p_pp.tile([128, 128], BF16, name="ps_vt", tag="pp")
                    nc.tensor.transpose(pt[:, :], vtt[:, sub * 128:(sub + 1) * 128],
                                        eye_sb[:, :])
                    v = p_vt.tile([128, D], BF16, name="v_t", tag="vt")
                    nc.scalar.copy(v[:, :], pt[:, :])
                    vb.append(v)

                # ---- attention for this chunk (scores transposed [k, q]),
                #      previous chunk's outproj groups interleaved ----
                at_c = [p_at.tile([D, QCH], BF16, name=f"atc{f}", tag="at")
                        for f in range(QH)]
                nk = KPQ * (tci + 1)
                pbk = min(5, nk - 1)
                for h in range(QH):
                    av = p_av.tile([128, QCH], F32, name="ps_av", tag="av")
                    acc = p_acc.tile([128, QCH], BF16, name="acc", tag="acc")
                    es = [None] * nk

                    def emit_av(kt):
                        e, lo, n = es[kt]
                        nc.tensor.matmul(av[:, lo:QCH], vb[kt][:, :], e[:, 0:n],
                                         start=(kt == 0), stop=(kt == nk - 1))

                    for kt in range(nk):
                        j = kt - KPQ * tci          # >=0 inside diagonal band
                        lo = max(j, 0) * 128        # first live column
                        n = QCH - lo
                        st = p_pp.tile([128, QCH], F32, name="ps_st", tag="pp")
                        nc.tensor.matmul(
                            st[:, 0:n], ktb[:, kt * 128:(kt + 1) * 128],
                            qtb_c[h][:, lo:QCH], start=True, stop=True)
                        if j >= 0:  # diagonal tile: triangular mask on 1st 128
                            nc.vector.tensor_add(st[:, 0:128], st[:, 0:128],
                                                 mask_sb[:, 0:128])
                        e = p_exp.tile([128, QCH], BF16, name="e_t", tag="exp")
                        nc.scalar.activation(e[:, 0:n], st[:, 0:n], EXP, scale=SC)
                        if kt == 0:
                            nc.vector.tensor_copy(acc[:, :], e[:, :])
                        else:
                            nc.vector.tensor_add(acc[:, lo:QCH], acc[:, lo:QCH],
                                                 e[:, 0:n])
                        es[kt] = (e, lo, n)
                        if kt == 1 and pend is not None:
                            pend2 = emit_norm_sum(pend)
                            pend = None
                        if kt == pbk and pend2 is not None:
                            emit_norm_apply(pend2)
                            pend2 = None
                        if kt >= PIPE:
                            emit_av(kt - PIPE)
                        if kt >= 2 and oq:
                            emit_op_group(oq.pop(0))
                    for kt in range(max(0, nk - PIPE), nk):
                        emit_av(kt)
                    pend = (acc, av, at_c[h])
                # chunk end: flush the last head's normalization into the
                # leftover outproj groups of the previous chunk
                pend2 = emit_norm_sum(pend)
                pend = None
                for i, g in enumerate(oq):
                    emit_op_group(g)
                    if i == 1 and pend2 is not None:
                        emit_norm_apply(pend2)
                        pend2 = None
                if pend2 is not None:
                    emit_norm_apply(pend2)
                    pend2 = None
                prev_op = (b, tci, at_c)

        # ---- tail: final chunk's outproj (its norms flushed at chunk end) ----
        for g in make_op_groups(*prev_op):
            emit_op_group(g)
    nc.finalize()  # Bacc: runs compile() (reg alloc, wait splitting) + freeze
    return nc


def shard_inputs(hidden_states, cos, sin, Wq, Wk, Wv, Wo, n_cores=N_CORES,
                 QH=4, D=128, QCH=512):
    """Host-side prep: transpose/tile/bf16-round everything per core."""
    bf16 = ml_dtypes.bfloat16
    B, S, HID = hidden_states.shape
    FS = QH * D
    KT = HID // 128
    TC = S // QCH

    hT = hidden_states.astype(bf16).transpose(0, 2, 1)           # [B, HID, S]
    ht_t = np.ascontiguousarray(
        hT.reshape(B, KT, 128, TC, QCH).transpose(0, 1, 3, 2, 4))
    trig = np.ascontiguousarray(np.stack(
        [cos.transpose(0, 2, 1), sin.transpose(0, 2, 1)], axis=1)
    ).astype(bf16)

    kk = np.arange(128)[:, None]
    cc = np.arange(QCH)[None, :]
    maskv = np.where(cc < kk, np.float32(NEG), np.float32(0.0))
    onesv = np.ones((128, 1), dtype=bf16)
    onesrv = np.ones((1, 128), dtype=bf16)
    eyev = np.eye(128, dtype=bf16)

    in_maps = []
    for i in range(n_cores):
        wq_i = Wq[i * FS:(i + 1) * FS, :].T.astype(bf16).reshape(KT, 128, FS)
        wk_i = Wk[i * D:(i + 1) * D, :].T.astype(bf16).reshape(KT, 128, D)
        wv_i = Wv[i * D:(i + 1) * D, :].T.astype(bf16).reshape(KT, 128, D)
        wo_i = Wo[:, i * FS:(i + 1) * FS].T.astype(bf16).reshape(QH, D, HID)
        in_maps.append(dict(ht=ht_t, trig=trig, wq=wq_i, wk=wk_i, wv=wv_i,
                            wo=wo_i, mask=maskv, ones=onesv, onesr=onesrv,
                            eye=eyev))
    return in_maps


_NC_CACHE = {}


def kernel(hidden_states, cos, sin, Wq, Wk, Wv, Wo, _trace=False):
    hidden_states = np.asarray(hidden_states)
    cos = np.asarray(cos)
    sin = np.asarray(sin)
    Wq, Wk, Wv, Wo = (np.asarray(a) for a in (Wq, Wk, Wv, Wo))
    B, S, HID = hidden_states.shape
    QCH = 512

    key = (B, S, HID)
    nc = _NC_CACHE.get(key)
    if nc is None:
        nc = _NC_CACHE[key] = build_core_kernel(B, S, HID)

    in_maps = shard_inputs(hidden_states, cos, sin, Wq, Wk, Wv, Wo)
    res = run_bass_kernel_spmd(nc, in_maps, core_ids=list(range(N_CORES)),
                               trace=_trace)
    kernel._last_results = res

    acc = res.results[0]["out"].astype(np.float32)
    for r in res.results[1:]:
        acc += r["out"].astype(np.float32)
    # [B, TT, HC, 128, QCH] -> [B, S, HID]
    TT = S // 128
    HC = HID // QCH
    full = acc.transpose(0, 1, 3, 2, 4).reshape(B, S, HID)
    return np.ascontiguousarray(full)


# revision 21
# speedup vs baseline: 1.1616x; 1.1616x over previous
# Trainium2 Bass kernel: Llama-style attention block (GQA + RoPE + causal),
# tensor-parallel across heads on 8 NeuronCores.
#
# Full-shape contract: kernel(**inputs) takes the unsharded numpy inputs and
# returns the full [B, S, HID] float32 output.
#
# Sharding strategy (per core i of 8):
#   - 4 query heads (rows i*512:(i+1)*512 of Wq) + 1 kv head (rows i*128.. of Wk/Wv)
#   - Wo is sharded row-wise (its columns i*512:(i+1)*512); each core emits a
#     partial [B,S,HID] product (bf16) which the host sums at gather time.
# All weights/activations are pre-transposed and pre-tiled on the host so the
# device kernel needs zero on-chip transposes of activations:
#   matmul(out[M,N], lhsT[K,M], rhs[K,N]) contracts over the partition dim K.
#
# Schedule: fully chunk-interleaved so the PE never idles long enough for the
# HAM clock gate to re-throttle, and so the scalar engine's exp work (which
# alone nearly matches the PE's score+AV matmul time) always has PE work to
# hide behind:
#   per 512-token chunk: QKV projection sweep -> RoPE -> V transpose ->
#   attention for the chunk's 4 heads, with (a) score matmuls software-
#   pipelined two steps ahead of the attn*V matmuls, (b) the softmax
#   normalization (all-bf16 sum + broadcast matmuls, fast-approx reciprocal)
#   deferred into the NEXT head's score stream, and (c) the PREVIOUS chunk's
#   output-projection groups interleaved one-per-kt into the score stream so
#   the PE stays busy when the exp stream is the rate limiter.
# Scores stay transposed [k, q]; softmax is unnormalized (scores are O(10) so
# exp is safe in fp32); the normalizer is folded in at the attention-output
# eviction.

import os
import sys
from contextlib import ExitStack

for _p in ("/opt/trn_rl_repo", "/root/.axon_site/_ro/trn_rl_repo"):
    if os.path.isdir(_p) and _p not in sys.path:
        sys.path.append(_p)

import ml_dtypes
import numpy as np

import concourse.bass as bass
import concourse.mybir as mybir
import concourse.tile as tile
from concourse import bacc
from concourse.bass_utils import run_bass_kernel_spmd

BF16 = mybir.dt.bfloat16
F32 = mybir.dt.float32
NEG = -1.0e9
N_CORES = 8


def build_core_kernel(B, S, HID, QH=4, D=128, QCH=512):
    """SPMD per-core program. QH query heads + 1 kv head per core.

    DRAM parameter layouts (host pre-tiles everything):
      ht   [B, KT, TC, 128, QCH]  bf16  hidden^T tiles: [b,kt,tc,i,j] = hidden[b, tc*QCH+j, kt*128+i]
      trig [B, 2, D, S]           bf16  cos^T / sin^T
      wq   [KT, 128, QH*D]        bf16  Wq_core^T tiles (contraction rows blocked by 128)
      wk   [KT, 128, D]           bf16
      wv   [KT, 128, D]           bf16
      wo   [QH, D, HID]           bf16  Wo_core^T rows blocked per head
      mask [128, QCH]             f32   additive causal triangle (cols 0:128), 0 beyond
      ones [128, 1]               bf16  partition-sum lhsT
      onesr [1, 128]              bf16  broadcast lhsT
      eye  [128, 128]             bf16
      out  [B, TT, HC, 128, QCH]  bf16  partial output tiles
    """
    FS = QH * D          # per-core feature slice of the qkv/attn space
    KT = HID // 128      # contraction tiles for projections
    TC = S // QCH        # 512-token chunks
    TT = S // 128        # 128-token tiles
    TPC = QCH // 128     # token tiles per chunk
    HC = HID // QCH      # output hid chunks
    KPQ = QCH // 128     # k-tiles per q-chunk (diagonal band width)
    HALF = D // 2
    PIPE = 2             # score->AV software pipeline depth
    SC = float(1.0 / np.sqrt(D))
    EXP = mybir.ActivationFunctionType.Exp

    # Bacc (not plain Bass): its compile pipeline splits multi-sem waits into
    # EventSemaphore instructions — the DMA DIRECT2D struct has one wait slot.
    nc = bacc.Bacc(None)
    ht = nc.declare_dram_parameter("ht", [B, KT, TC, 128, QCH], BF16, isOutput=False)
    trig = nc.declare_dram_parameter("trig", [B, 2, D, S], BF16, isOutput=False)
    wq = nc.declare_dram_parameter("wq", [KT, 128, FS], BF16, isOutput=False)
    wk = nc.declare_dram_parameter("wk", [KT, 128, D], BF16, isOutput=False)
    wv = nc.declare_dram_parameter("wv", [KT, 128, D], BF16, isOutput=False)
    wo = nc.declare_dram_parameter("wo", [QH, D, HID], BF16, isOutput=False)
    mask = nc.declare_dram_parameter("mask", [128, QCH], F32, isOutput=False)
    ones = nc.declare_dram_parameter("ones", [128, 1], BF16, isOutput=False)
    onesr = nc.declare_dram_parameter("onesr", [1, 128], BF16, isOutput=False)
    eye = nc.declare_dram_parameter("eye", [128, 128], BF16, isOutput=False)
    out = nc.declare_dram_parameter("out", [B, TT, HC, 128, QCH], BF16, isOutput=True)

    with ExitStack() as ctx:
        tc = ctx.enter_context(tile.TileContext(nc))
        pool = lambda name, bufs, space=None: ctx.enter_context(
            tc.tile_pool(name=name, bufs=bufs, **({"space": space} if space else {}))
        )
        p_w = pool("p_w", 1)          # weights + constants, loaded once
        p_ht = pool("p_ht", 35)       # streamed hidden^T tiles (full chunk + prefetch)
        p_qt = pool("p_qt", 8)        # per-chunk per-head Q^T [D, QCH] bf16
        p_kt = pool("p_kt", 1)        # K^T [D, S] bf16 (persistent per batch)
        p_vtt = pool("p_vtt", 3)      # V^T staging before transpose
        p_vt = pool("p_vt", S // 128 + 4)  # V tiles [128 tok, D]
        p_at = pool("p_at", 8)        # per-chunk per-head attn^T [D, QCH] bf16
        p_exp = pool("p_exp", 7)      # exp(score) tiles bf16
        p_rt = pool("p_rt", 1)        # rope temps f32
        p_acc = pool("p_acc", 2)      # softmax-sum accumulators bf16
        p_sums = pool("p_sums", 2)    # softmax sums row bf16
        p_rbc = pool("p_rbc", 2)      # broadcast reciprocal f32
        p_ost = pool("p_ost", 4)      # output staging bf16
        p_pp = pool("p_pp", 5, "PSUM")    # general [128, QCH] psum ring
        p_av = pool("p_av", 2, "PSUM")    # attn-v accumulators (+proj V)
        p_sm = pool("p_sm", 1, "PSUM")    # softmax sum rows

        wq_sb = p_w.tile([128, KT, FS], BF16, name="wq_sb")
        wk_sb = p_w.tile([128, KT, D], BF16, name="wk_sb")
        wv_sb = p_w.tile([128, KT, D], BF16, name="wv_sb")
        wo_sb = p_w.tile([128, QH, HID], BF16, name="wo_sb")
        mask_sb = p_w.tile([128, QCH], F32, name="mask_sb")
        ones_sb = p_w.tile([128, 1], BF16, name="ones_sb")
        onesr_sb = p_w.tile([1, 128], BF16, name="onesr_sb")
        eye_sb = p_w.tile([128, 128], BF16, name="eye_sb")
        cos_sb = p_w.tile([D, B, S], BF16, name="cos_sb")
        sin_sb = p_w.tile([D, B, S], BF16, name="sin_sb")
        # Startup order: tiny constants first, then per-kt weights split
        # across the two DMA queues so the first projection sweep's operands
        # stay ahead of the PE's ~1.1 us/kt consumption rate. cos/sin are
        # needed ~35 us in (first RoPE eviction), wo at ~90 us (first outproj).
        nc.scalar.dma_start(out=mask_sb[:, :], in_=mask[:, :])
        nc.scalar.dma_start(out=ones_sb[:, :], in_=ones[:, :])
        nc.scalar.dma_start(out=onesr_sb[:, :], in_=onesr[:, :])
        nc.scalar.dma_start(out=eye_sb[:, :], in_=eye[:, :])
        for kt in range(KT):
            nc.scalar.dma_start(out=wq_sb[:, kt, :], in_=wq[kt])
            nc.sync.dma_start(out=wk_sb[:, kt, :], in_=wk[kt])
            nc.sync.dma_start(out=wv_sb[:, kt, :], in_=wv[kt])
        for b in range(B):
            nc.scalar.dma_start(out=cos_sb[:, b, :], in_=trig[b, 0])
            nc.scalar.dma_start(out=sin_sb[:, b, :], in_=trig[b, 1])
        for f in range(QH):
            nc.scalar.dma_start(out=wo_sb[:, f, :], in_=wo[f])

        def rope_evict(b, dst, ps, tci):
            # dst[:, :] = ps * cos + rotate_half(ps) * sin  (write bf16, chunk tci)
            # bf16 temporaries: DVE runs 2 elems/cycle for 16-bit vs 1 for f32.
            sl = slice(tci * QCH, (tci + 1) * QCH)
            cs = cos_sb[:, b, sl]
            sn = sin_sb[:, b, sl]
            t1 = p_rt.tile([128, QCH], BF16, name="rt1", tag="rt1")
            t2 = p_rt.tile([128, QCH], BF16, name="rt2", tag="rt2")
            nc.vector.tensor_mul(t1[:, :], ps[:, :], cs)
            nc.vector.tensor_mul(t2[0:HALF, :], ps[HALF:D, :], sn[0:HALF, :])
            nc.vector.tensor_mul(t2[HALF:D, :], ps[0:HALF, :], sn[HALF:D, :])
            nc.vector.tensor_sub(dst[0:HALF, :], t1[0:HALF, :], t2[0:HALF, :])
            nc.vector.tensor_add(dst[HALF:D, :], t1[HALF:D, :], t2[HALF:D, :])

        # Deferred softmax normalization: `pend` carries (acc, av, at_dst) of
        # the previous head; its PE ops (sum + broadcast matmuls, all bf16)
        # are issued inside the NEXT head's score stream so their
        # vector/scalar dependencies are long satisfied.
        def emit_norm_sum(pend):
            acc, av, at_dst = pend
            sm = p_sm.tile([1, QCH], F32, name="ps_sm", tag="sm")
            nc.tensor.matmul(sm[:, :], ones_sb[:, :], acc[:, :],
                             start=True, stop=True)
            sms = p_sums.tile([1, QCH], BF16, name="sms", tag="sums")
            nc.scalar.copy(sms[:, :], sm[:, :])
            return (av, at_dst, sms)

        def emit_norm_apply(pend2):
            av, at_dst, sms = pend2
            # broadcast sums down partitions via K=1 outer product
            pb = p_pp.tile([128, QCH], F32, name="ps_pb", tag="pp")
            nc.tensor.matmul(pb[:, :], onesr_sb[:, :], sms[:, :],
                             start=True, stop=True)
            rb = p_rbc.tile([128, QCH], F32, name="rb", tag="rb")
            nc.vector.reciprocal_approx_fast(rb[:, :], pb[:, :])
            nc.vector.tensor_mul(at_dst[:, :], av[:, :], rb[:, :])

        # Output projection emitters for one chunk (32 (tt,hc) groups); popped
        # one-per-kt into the next chunk's attention score stream.
        def emit_op_group(g):
            bb, tt, tt4, hc, atp = g
            csl = slice(tt4 * 128, (tt4 + 1) * 128)
            pw = p_pp.tile([128, QCH], F32, name="ps_wo", tag="pp")
            for f in range(QH):
                nc.tensor.matmul(pw[:, :], atp[f][:, csl],
                                 wo_sb[:, f, hc * QCH:(hc + 1) * QCH],
                                 start=(f == 0), stop=(f == QH - 1))
            o = p_ost.tile([128, QCH], BF16, name="o_t", tag="ost")
            if hc % 2 == 0:
                nc.vector.tensor_copy(o[:, :], pw[:, :])
            else:
                nc.scalar.copy(o[:, :], pw[:, :])
            nc.sync.dma_start(out=out[bb, tt, hc], in_=o[:, :])

        def make_op_groups(bb, tcip, atp):
            return [(bb, tcip * TPC + tt4, tt4, hc, atp)
                    for tt4 in range(TPC) for hc in range(HC)]

        pend = None       # deferred normalization state (crosses chunks)
        pend2 = None
        prev_op = None    # (b, tci, at_c) of the chunk awaiting outproj

        for b in range(B):
            vb = []       # V tiles [128 tok, D], global k-tile index
            ktb = p_kt.tile([D, S], BF16, name="ktb", tag="kt")
            for tci in range(TC):
                # ---- projection sweep 1: Q heads + K accumulate per kt ----
                qps = [p_pp.tile([128, QCH], F32, name=f"ps_q{f}", tag="pp")
                       for f in range(QH)]
                kps = p_pp.tile([128, QCH], F32, name="ps_k", tag="pp")
                hts = []
                for kt in range(KT):
                    t = p_ht.tile([128, QCH], BF16, name="ht_t", tag="ht")
                    nc.sync.dma_start(out=t[:, :], in_=ht[b, kt, tci])
                    hts.append(t)
                    st_, sp_ = (kt == 0), (kt == KT - 1)
                    for f in range(QH):
                        nc.tensor.matmul(qps[f][:, :],
                                         wq_sb[:, kt, f * D:(f + 1) * D],
                                         t[:, :], start=st_, stop=sp_)
                    nc.tensor.matmul(kps[:, :], wk_sb[:, kt, :], t[:, :],
                                     start=st_, stop=sp_)
                qtb_c = [p_qt.tile([D, QCH], BF16, name=f"qtc{f}", tag="qt")
                         for f in range(QH)]
                ktb_c = ktb[:, tci * QCH:(tci + 1) * QCH]
                # RoPE order on the DVE: K first (the score matmuls' stationary
                # operand), then q0/q1; vtt is copied mid-stream so the PE's
                # transposes unblock right after sweep 2; q2/q3 follow (their
                # heads start much later).
                rope_evict(b, ktb_c, kps, tci)
                for f in range(QH):
                    rope_evict(b, qtb_c[f], qps[f], tci)
                # ---- sweep 2: V^T accumulation, then PE-transpose ----
                vps = p_av.tile([128, QCH], F32, name="ps_v", tag="av")
                for kt in range(KT):
                    nc.tensor.matmul(vps[:, :], wv_sb[:, kt, :], hts[kt][:, :],
                                     start=(kt == 0), stop=(kt == KT - 1))
                vtt = p_vtt.tile([128, QCH], BF16, name="vtt", tag="vtt")
                nc.vector.tensor_copy(vtt[:, :], vps[:, :])
                for sub in range(TPC):
                    pt = p_pp.tile([128, 128], BF16, name="ps_vt", tag="pp")
                    nc.tensor.transpose(pt[:, :], vtt[:, sub * 128:(sub + 1) * 128],
                                        eye_sb[:, :])
                    v = p_vt.tile([128, D], BF16, name="v_t", tag="vt")
                    nc.scalar.copy(v[:, :], pt[:, :])
                    vb.append(v)

                # ---- attention for this chunk (scores transposed [k, q]),
                #      previous chunk's outproj groups interleaved ----
                oq = make_op_groups(*prev_op) if prev_op else []
                at_c = [p_at.tile([D, QCH], BF16, name=f"atc{f}", tag="at")
                        for f in range(QH)]
                nk = KPQ * (tci + 1)
                for h in range(QH):
                    av = p_av.tile([128, QCH], F32, name="ps_av", tag="av")
                    acc = p_acc.tile([128, QCH], BF16, name="acc", tag="acc")
                    es = [None] * nk

                    def emit_av(kt):
                        e, lo, n = es[kt]
                        nc.tensor.matmul(av[:, lo:QCH], vb[kt][:, :], e[:, 0:n],
                                         start=(kt == 0), stop=(kt == nk - 1))

                    for kt in range(nk):
                        j = kt - KPQ * tci          # >=0 inside diagonal band
                        lo = max(j, 0) * 128        # first live column
                        n = QCH - lo
                        st = p_pp.tile([128, QCH], F32, name="ps_st", tag="pp")
                        nc.tensor.matmul(
                            st[:, 0:n], ktb[:, kt * 128:(kt + 1) * 128],
                            qtb_c[h][:, lo:QCH], start=True, stop=True)
                        if j >= 0:  # diagonal tile: triangular mask on 1st 128
                            nc.vector.tensor_add(st[:, 0:128], st[:, 0:128],
                                                 mask_sb[:, 0:128])
                        e = p_exp.tile([128, QCH], BF16, name="e_t", tag="exp")
                        nc.scalar.activation(e[:, 0:n], st[:, 0:n], EXP, scale=SC)
                        if kt == 0:
                            nc.vector.tensor_copy(acc[:, :], e[:, :])
                        else:
                            nc.vector.tensor_add(acc[:, lo:QCH], acc[:, lo:QCH],
                                                 e[:, 0:n])
                        es[kt] = (e, lo, n)
                        if kt == 1 and pend is not None:
                            pend2 = emit_norm_sum(pend)
                            pend = None
                        if kt == 2 and pend2 is not None:
                            emit_norm_apply(pend2)
                            pend2 = None
                        if kt >= PIPE:
                            emit_av(kt - PIPE)
                        if kt >= 3 and oq:
                            emit_op_group(oq.pop(0))
                    for kt in range(max(0, nk - PIPE), nk):
                        emit_av(kt)
                    pend = (acc, av, at_c[h])
                # leftover outproj groups of the previous chunk
                for g in oq:
                    emit_op_group(g)
                prev_op = (b, tci, at_c)

        # ---- tail: flush the last head's normalization + final outproj ----
        pend2 = emit_norm_sum(pend)
        pend = None
        emit_norm_apply(pend2)
        pend2 = None
        for g in make_op_groups(*prev_op):
            emit_op_group(g)
    nc.finalize()  # Bacc: runs compile() (reg alloc, wait splitting) + freeze
    return nc


def shard_inputs(hidden_states, cos, sin, Wq, Wk, Wv, Wo, n_cores=N_CORES,
                 QH=4, D=128, QCH=512):
    """Host-side prep: transpose/tile/bf16-round everything per core."""
    bf16 = ml_dtypes.bfloat16
    B, S, HID = hidden_states.shape
    FS = QH * D
    KT = HID // 128
    TC = S // QCH

    hT = hidden_states.astype(bf16).transpose(0, 2, 1)           # [B, HID, S]
    ht_t = np.ascontiguousarray(
        hT.reshape(B, KT, 128, TC, QCH).transpose(0, 1, 3, 2, 4))
    trig = np.ascontiguousarray(np.stack(
        [cos.transpose(0, 2, 1), sin.transpose(0, 2, 1)], axis=1)
    ).astype(bf16)

    kk = np.arange(128)[:, None]
    cc = np.arange(QCH)[None, :]
    maskv = np.where(cc < kk, np.float32(NEG), np.float32(0.0))
    onesv = np.ones((128, 1), dtype=bf16)
    onesrv = np.ones((1, 128), dtype=bf16)
    eyev = np.eye(128, dtype=bf16)

    in_maps = []
    for i in range(n_cores):
        wq_i = Wq[i * FS:(i + 1) * FS, :].T.astype(bf16).reshape(KT, 128, FS)
        wk_i = Wk[i * D:(i + 1) * D, :].T.astype(bf16).reshape(KT, 128, D)
        wv_i = Wv[i * D:(i + 1) * D, :].T.astype(bf16).reshape(KT, 128, D)
        wo_i = Wo[:, i * FS:(i + 1) * FS].T.astype(bf16).reshape(QH, D, HID)
        in_maps.append(dict(ht=ht_t, trig=trig, wq=wq_i, wk=wk_i, wv=wv_i,
                            wo=wo_i, mask=maskv, ones=onesv, onesr=onesrv,
                            eye=eyev))
    return in_maps


_NC_CACHE = {}


def kernel(hidden_states, cos, sin, Wq, Wk, Wv, Wo, _trace=False):
    hidden_states = np.asarray(hidden_states)
    cos = np.asarray(cos)
    sin = np.asarray(sin)
    Wq, Wk, Wv, Wo = (np.asarray(a) for a in (Wq, Wk, Wv, Wo))
    B, S, HID = hidden_states.shape
    QCH = 512

    key = (B, S, HID)
    nc = _NC_CACHE.get(key)
    if nc is None:
        nc = _NC_CACHE[key] = build_core_kernel(B, S, HID)

    in_maps = shard_inputs(hidden_states, cos, sin, Wq, Wk, Wv, Wo)
    res = run_bass_kernel_spmd(nc, in_maps, core_ids=list(range(N_CORES)),
                               trace=_trace)
    kernel._last_results = res

    acc = res.results[0]["out"].astype(np.float32)
    for r in res.results[1:]:
        acc += r["out"].astype(np.float32)
    # [B, TT, HC, 128, QCH] -> [B, S, HID]
    TT = S // 128
    HC = HID // QCH
    full = acc.transpose(0, 1, 3, 2, 4).reshape(B, S, HID)
    return np.ascontiguousarray(full)


# revision 22
# speedup vs baseline: 1.2131x; 1.0443x over previous
# Trainium2 Bass kernel: Llama-style attention block (GQA + RoPE + causal),
# tensor-parallel across heads on 8 NeuronCores.
#
# Full-shape contract: kernel(**inputs) takes the unsharded numpy inputs and
# returns the full [B, S, HID] float32 output.
#
# Sharding strategy (per core i of 8):
#   - 4 query heads (rows i*512:(i+1)*512 of Wq) + 1 kv head (rows i*128.. of Wk/Wv)
#   - Wo is sharded row-wise (its columns i*512:(i+1)*512); each core emits a
#     partial [B,S,HID] product (bf16) which the host sums at gather time.
# All weights/activations are pre-transposed and pre-tiled on the host so the
# device kernel needs zero on-chip transposes of activations:
#   matmul(out[M,N], lhsT[K,M], rhs[K,N]) contracts over the partition dim K.
#
# Schedule: fully chunk-interleaved so the PE never idles long enough for the
# HAM clock gate to re-throttle, and so the scalar engine's exp work (which
# alone nearly matches the PE's score+AV matmul time) always has PE work to
# hide behind:
#   per 512-token chunk: QKV projection sweep -> RoPE -> V transpose ->
#   attention for the chunk's 4 heads, with (a) score matmuls software-
#   pipelined two steps ahead of the attn*V matmuls, (b) the softmax
#   normalization (all-bf16 sum + broadcast matmuls, fast-approx reciprocal)
#   deferred into the NEXT head's score stream, and (c) the PREVIOUS chunk's
#   output-projection groups interleaved one-per-kt into the score stream so
#   the PE stays busy when the exp stream is the rate limiter.
# Scores stay transposed [k, q]; softmax is unnormalized (scores are O(10) so
# exp is safe in fp32); the normalizer is folded in at the attention-output
# eviction.

import os
import sys
from contextlib import ExitStack

for _p in ("/opt/trn_rl_repo", "/root/.axon_site/_ro/trn_rl_repo"):
    if os.path.isdir(_p) and _p not in sys.path:
        sys.path.append(_p)

import ml_dtypes
import numpy as np

import concourse.bass as bass
import concourse.mybir as mybir
import concourse.tile as tile
from concourse import bacc
from concourse.bass_utils import run_bass_kernel_spmd

BF16 = mybir.dt.bfloat16
F32 = mybir.dt.float32
NEG = -1.0e9
N_CORES = 8


def build_core_kernel(B, S, HID, QH=4, D=128, QCH=512):
    """SPMD per-core program. QH query heads + 1 kv head per core.

    DRAM parameter layouts (host pre-tiles everything):
      ht   [B, KT, TC, 128, QCH]  bf16  hidden^T tiles: [b,kt,tc,i,j] = hidden[b, tc*QCH+j, kt*128+i]
      trig [B, 2, D, S]           bf16  cos^T / sin^T
      wq   [KT, 128, QH*D]        bf16  Wq_core^T tiles (contraction rows blocked by 128)
      wk   [KT, 128, D]           bf16
      wv   [KT, 128, D]           bf16
      wo   [QH, D, HID]           bf16  Wo_core^T rows blocked per head
      mask [128, QCH]             f32   additive causal triangle (cols 0:128), 0 beyond
      ones [128, 1]               bf16  partition-sum lhsT
      onesr [1, 128]              bf16  broadcast lhsT
      eye  [128, 128]             bf16
      out  [B, TT, HC, 128, QCH]  bf16  partial output tiles
    """
    FS = QH * D          # per-core feature slice of the qkv/attn space
    KT = HID // 128      # contraction tiles for projections
    TC = S // QCH        # 512-token chunks
    TT = S // 128        # 128-token tiles
    TPC = QCH // 128     # token tiles per chunk
    HC = HID // QCH      # output hid chunks
    KPQ = QCH // 128     # k-tiles per q-chunk (diagonal band width)
    HALF = D // 2
    PIPE = 2             # score->AV software pipeline depth
    SC = float(1.0 / np.sqrt(D))
    EXP = mybir.ActivationFunctionType.Exp

    # Bacc (not plain Bass): its compile pipeline splits multi-sem waits into
    # EventSemaphore instructions — the DMA DIRECT2D struct has one wait slot.
    nc = bacc.Bacc(None)
    ht = nc.declare_dram_parameter("ht", [B, KT, TC, 128, QCH], BF16, isOutput=False)
    trig = nc.declare_dram_parameter("trig", [B, 2, D, S], BF16, isOutput=False)
    wq = nc.declare_dram_parameter("wq", [KT, 128, FS], BF16, isOutput=False)
    wk = nc.declare_dram_parameter("wk", [KT, 128, D], BF16, isOutput=False)
    wv = nc.declare_dram_parameter("wv", [KT, 128, D], BF16, isOutput=False)
    wo = nc.declare_dram_parameter("wo", [QH, D, HID], BF16, isOutput=False)
    mask = nc.declare_dram_parameter("mask", [128, QCH], F32, isOutput=False)
    ones = nc.declare_dram_parameter("ones", [128, 1], BF16, isOutput=False)
    onesr = nc.declare_dram_parameter("onesr", [1, 128], BF16, isOutput=False)
    eye = nc.declare_dram_parameter("eye", [128, 128], BF16, isOutput=False)
    out = nc.declare_dram_parameter("out", [B, TT, HC, 128, QCH], BF16, isOutput=True)

    with ExitStack() as ctx:
        tc = ctx.enter_context(tile.TileContext(nc))
        pool = lambda name, bufs, space=None: ctx.enter_context(
            tc.tile_pool(name=name, bufs=bufs, **({"space": space} if space else {}))
        )
        p_w = pool("p_w", 1)          # weights + constants, loaded once
        p_ht = pool("p_ht", 35)       # streamed hidden^T tiles (full chunk + prefetch)
        p_qt = pool("p_qt", 8)        # per-chunk per-head Q^T [D, QCH] bf16
        p_kt = pool("p_kt", 1)        # K^T [D, S] bf16 (persistent per batch)
        p_vtt = pool("p_vtt", 3)      # V^T staging before transpose
        p_vt = pool("p_vt", S // 128 + 4)  # V tiles [128 tok, D]
        p_at = pool("p_at", 8)        # per-chunk per-head attn^T [D, QCH] bf16
        p_exp = pool("p_exp", 7)      # exp(score) tiles bf16
        p_rt = pool("p_rt", 1)        # rope temps f32
        p_acc = pool("p_acc", 2)      # softmax-sum accumulators bf16
        p_sums = pool("p_sums", 2)    # softmax sums row bf16
        p_rbc = pool("p_rbc", 2)      # broadcast reciprocal f32
        p_ost = pool("p_ost", 4)      # output staging bf16
        p_pp = pool("p_pp", 5, "PSUM")    # general [128, QCH] psum ring
        p_av = pool("p_av", 2, "PSUM")    # attn-v accumulators (+proj V)
        p_sm = pool("p_sm", 1, "PSUM")    # softmax sum rows

        wq_sb = p_w.tile([128, KT, FS], BF16, name="wq_sb")
        wk_sb = p_w.tile([128, KT, D], BF16, name="wk_sb")
        wv_sb = p_w.tile([128, KT, D], BF16, name="wv_sb")
        wo_sb = p_w.tile([128, QH, HID], BF16, name="wo_sb")
        mask_sb = p_w.tile([128, QCH], F32, name="mask_sb")
        ones_sb = p_w.tile([128, 1], BF16, name="ones_sb")
        onesr_sb = p_w.tile([1, 128], BF16, name="onesr_sb")
        eye_sb = p_w.tile([128, 128], BF16, name="eye_sb")
        cos_sb = p_w.tile([D, B, S], BF16, name="cos_sb")
        sin_sb = p_w.tile([D, B, S], BF16, name="sin_sb")
        # Startup order: tiny constants first, then per-kt weights split
        # across the two DMA queues so the first projection sweep's operands
        # stay ahead of the PE's ~1.1 us/kt consumption rate. cos/sin are
        # needed ~35 us in (first RoPE eviction), wo at ~90 us (first outproj).
        nc.scalar.dma_start(out=mask_sb[:, :], in_=mask[:, :])
        nc.scalar.dma_start(out=ones_sb[:, :], in_=ones[:, :])
        nc.scalar.dma_start(out=onesr_sb[:, :], in_=onesr[:, :])
        nc.scalar.dma_start(out=eye_sb[:, :], in_=eye[:, :])
        for kt in range(KT):
            nc.scalar.dma_start(out=wq_sb[:, kt, :], in_=wq[kt])
            nc.sync.dma_start(out=wk_sb[:, kt, :], in_=wk[kt])
        for b in range(B):
            nc.scalar.dma_start(out=cos_sb[:, b, :], in_=trig[b, 0])
            nc.scalar.dma_start(out=sin_sb[:, b, :], in_=trig[b, 1])
        for kt in range(KT):
            nc.scalar.dma_start(out=wv_sb[:, kt, :], in_=wv[kt])
        for f in range(QH):
            nc.scalar.dma_start(out=wo_sb[:, f, :], in_=wo[f])

        def rope_evict(b, dst, ps, tci):
            # dst[:, :] = ps * cos + rotate_half(ps) * sin  (write bf16, chunk tci)
            # bf16 temporaries: DVE runs 2 elems/cycle for 16-bit vs 1 for f32.
            sl = slice(tci * QCH, (tci + 1) * QCH)
            cs = cos_sb[:, b, sl]
            sn = sin_sb[:, b, sl]
            t1 = p_rt.tile([128, QCH], BF16, name="rt1", tag="rt1")
            t2 = p_rt.tile([128, QCH], BF16, name="rt2", tag="rt2")
            nc.vector.tensor_mul(t1[:, :], ps[:, :], cs)
            nc.vector.tensor_mul(t2[0:HALF, :], ps[HALF:D, :], sn[0:HALF, :])
            nc.vector.tensor_mul(t2[HALF:D, :], ps[0:HALF, :], sn[HALF:D, :])
            nc.vector.tensor_sub(dst[0:HALF, :], t1[0:HALF, :], t2[0:HALF, :])
            nc.vector.tensor_add(dst[HALF:D, :], t1[HALF:D, :], t2[HALF:D, :])

        # Deferred softmax normalization: `pend` carries (acc, av, at_dst) of
        # the previous head; its PE ops (sum + broadcast matmuls, all bf16)
        # are issued inside the NEXT head's score stream so their
        # vector/scalar dependencies are long satisfied.
        def emit_norm_sum(pend):
            acc, av, at_dst = pend
            sm = p_sm.tile([1, QCH], F32, name="ps_sm", tag="sm")
            nc.tensor.matmul(sm[:, :], ones_sb[:, :], acc[:, :],
                             start=True, stop=True)
            sms = p_sums.tile([1, QCH], BF16, name="sms", tag="sums")
            nc.scalar.copy(sms[:, :], sm[:, :])
            return (av, at_dst, sms)

        def emit_norm_apply(pend2):
            av, at_dst, sms = pend2
            # broadcast sums down partitions via K=1 outer product
            pb = p_pp.tile([128, QCH], F32, name="ps_pb", tag="pp")
            nc.tensor.matmul(pb[:, :], onesr_sb[:, :], sms[:, :],
                             start=True, stop=True)
            rb = p_rbc.tile([128, QCH], F32, name="rb", tag="rb")
            nc.vector.reciprocal_approx_fast(rb[:, :], pb[:, :])
            nc.vector.tensor_mul(at_dst[:, :], av[:, :], rb[:, :])

        # Output projection emitters for one chunk (32 (tt,hc) groups); popped
        # one-per-kt into the next chunk's attention score stream.
        def emit_op_group(g):
            bb, tt, tt4, hc, atp = g
            csl = slice(tt4 * 128, (tt4 + 1) * 128)
            pw = p_pp.tile([128, QCH], F32, name="ps_wo", tag="pp")
            for f in range(QH):
                nc.tensor.matmul(pw[:, :], atp[f][:, csl],
                                 wo_sb[:, f, hc * QCH:(hc + 1) * QCH],
                                 start=(f == 0), stop=(f == QH - 1))
            o = p_ost.tile([128, QCH], BF16, name="o_t", tag="ost")
            if hc % 2 == 0:
                nc.vector.tensor_copy(o[:, :], pw[:, :])
            else:
                nc.scalar.copy(o[:, :], pw[:, :])
            nc.sync.dma_start(out=out[bb, tt, hc], in_=o[:, :])

        def make_op_groups(bb, tcip, atp):
            return [(bb, tcip * TPC + tt4, tt4, hc, atp)
                    for tt4 in range(TPC) for hc in range(HC)]

        pend = None       # deferred normalization state (crosses chunks)
        pend2 = None
        prev_op = None    # (b, tci, at_c) of the chunk awaiting outproj

        for b in range(B):
            vb = []       # V tiles [128 tok, D], global k-tile index
            ktb = p_kt.tile([D, S], BF16, name="ktb", tag="kt")
            for tci in range(TC):
                # ---- projection sweep 1: Q heads + K accumulate per kt ----
                qps = [p_pp.tile([128, QCH], F32, name=f"ps_q{f}", tag="pp")
                       for f in range(QH)]
                kps = p_sm.tile([128, QCH], F32, name="ps_k", tag="sm")
                hts = []
                for kt in range(KT):
                    t = p_ht.tile([128, QCH], BF16, name="ht_t", tag="ht")
                    nc.sync.dma_start(out=t[:, :], in_=ht[b, kt, tci])
                    hts.append(t)
                    st_, sp_ = (kt == 0), (kt == KT - 1)
                    for f in range(QH):
                        nc.tensor.matmul(qps[f][:, :],
                                         wq_sb[:, kt, f * D:(f + 1) * D],
                                         t[:, :], start=st_, stop=sp_)
                    nc.tensor.matmul(kps[:, :], wk_sb[:, kt, :], t[:, :],
                                     start=st_, stop=sp_)
                qtb_c = [p_qt.tile([D, QCH], BF16, name=f"qtc{f}", tag="qt")
                         for f in range(QH)]
                ktb_c = ktb[:, tci * QCH:(tci + 1) * QCH]
                # RoPE order on the DVE: K first (the score matmuls' stationary
                # operand), then q0/q1; vtt is copied mid-stream so the PE's
                # transposes unblock right after sweep 2; q2/q3 follow (their
                # heads start much later).
                rope_evict(b, ktb_c, kps, tci)
                for f in range(QH):
                    rope_evict(b, qtb_c[f], qps[f], tci)
                # ---- sweep 2: V^T accumulation, then PE-transpose ----
                vps = p_av.tile([128, QCH], F32, name="ps_v", tag="av")
                for kt in range(KT):
                    nc.tensor.matmul(vps[:, :], wv_sb[:, kt, :], hts[kt][:, :],
                                     start=(kt == 0), stop=(kt == KT - 1))
                vtt = p_vtt.tile([128, QCH], BF16, name="vtt", tag="vtt")
                nc.vector.tensor_copy(vtt[:, :], vps[:, :])
                for sub in range(TPC):
                    pt = p_pp.tile([128, 128], BF16, name="ps_vt", tag="pp")
                    nc.tensor.transpose(pt[:, :], vtt[:, sub * 128:(sub + 1) * 128],
                                        eye_sb[:, :])
                    v = p_vt.tile([128, D], BF16, name="v_t", tag="vt")
                    nc.scalar.copy(v[:, :], pt[:, :])
                    vb.append(v)

                # ---- attention for this chunk (scores transposed [k, q]),
                #      previous chunk's outproj groups interleaved ----
                oq = make_op_groups(*prev_op) if prev_op else []
                at_c = [p_at.tile([D, QCH], BF16, name=f"atc{f}", tag="at")
                        for f in range(QH)]
                nk = KPQ * (tci + 1)
                for h in range(QH):
                    av = p_av.tile([128, QCH], F32, name="ps_av", tag="av")
                    acc = p_acc.tile([128, QCH], BF16, name="acc", tag="acc")
                    es = [None] * nk

                    def emit_av(kt):
                        e, lo, n = es[kt]
                        nc.tensor.matmul(av[:, lo:QCH], vb[kt][:, :], e[:, 0:n],
                                         start=(kt == 0), stop=(kt == nk - 1))

                    for kt in range(nk):
                        j = kt - KPQ * tci          # >=0 inside diagonal band
                        lo = max(j, 0) * 128        # first live column
                        n = QCH - lo
                        st = p_pp.tile([128, QCH], F32, name="ps_st", tag="pp")
                        nc.tensor.matmul(
                            st[:, 0:n], ktb[:, kt * 128:(kt + 1) * 128],
                            qtb_c[h][:, lo:QCH], start=True, stop=True)
                        if j >= 0:  # diagonal tile: triangular mask on 1st 128
                            nc.vector.tensor_add(st[:, 0:128], st[:, 0:128],
                                                 mask_sb[:, 0:128])
                        e = p_exp.tile([128, QCH], BF16, name="e_t", tag="exp")
                        nc.scalar.activation(e[:, 0:n], st[:, 0:n], EXP, scale=SC)
                        if kt == 0:
                            nc.vector.tensor_copy(acc[:, :], e[:, :])
                        else:
                            nc.vector.tensor_add(acc[:, lo:QCH], acc[:, lo:QCH],
                                                 e[:, 0:n])
                        es[kt] = (e, lo, n)
                        if kt == 1 and pend is not None:
                            pend2 = emit_norm_sum(pend)
                            pend = None
                        if kt == 2 and pend2 is not None:
                            emit_norm_apply(pend2)
                            pend2 = None
                        if kt >= PIPE:
                            emit_av(kt - PIPE)
                        if kt >= 3 and oq:
                            emit_op_group(oq.pop(0))
                    for kt in range(max(0, nk - PIPE), nk):
                        emit_av(kt)
                    pend = (acc, av, at_c[h])
                # leftover outproj groups of the previous chunk
                for g in oq:
                    emit_op_group(g)
                prev_op = (b, tci, at_c)

        # ---- tail: flush the last head's normalization + final outproj ----
        pend2 = emit_norm_sum(pend)
        pend = None
        emit_norm_apply(pend2)
        pend2 = None
        for g in make_op_groups(*prev_op):
            emit_op_group(g)
    nc.finalize()  # Bacc: runs compile() (reg alloc, wait splitting) + freeze
    return nc


def shard_inputs(hidden_states, cos, sin, Wq, Wk, Wv, Wo, n_cores=N_CORES,
                 QH=4, D=128, QCH=512):
    """Host-side prep: transpose/tile/bf16-round everything per core."""
    bf16 = ml_dtypes.bfloat16
    B, S, HID = hidden_states.shape
    FS = QH * D
    KT = HID // 128
    TC = S // QCH

    hT = hidden_states.astype(bf16).transpose(0, 2, 1)           # [B, HID, S]
    ht_t = np.ascontiguousarray(
        hT.reshape(B, KT, 128, TC, QCH).transpose(0, 1, 3, 2, 4))
    trig = np.ascontiguousarray(np.stack(
        [cos.transpose(0, 2, 1), sin.transpose(0, 2, 1)], axis=1)
    ).astype(bf16)

    kk = np.arange(128)[:, None]
    cc = np.arange(QCH)[None, :]
    maskv = np.where(cc < kk, np.float32(NEG), np.float32(0.0))
    onesv = np.ones((128, 1), dtype=bf16)
    onesrv = np.ones((1, 128), dtype=bf16)
    eyev = np.eye(128, dtype=bf16)

    in_maps = []
    for i in range(n_cores):
        wq_i = Wq[i * FS:(i + 1) * FS, :].T.astype(bf16).reshape(KT, 128, FS)
        wk_i = Wk[i * D:(i + 1) * D, :].T.astype(bf16).reshape(KT, 128, D)
        wv_i = Wv[i * D:(i + 1) * D, :].T.astype(bf16).reshape(KT, 128, D)
        wo_i = Wo[:, i * FS:(i + 1) * FS].T.astype(bf16).reshape(QH, D, HID)
        in_maps.append(dict(ht=ht_t, trig=trig, wq=wq_i, wk=wk_i, wv=wv_i,
                            wo=wo_i, mask=maskv, ones=onesv, onesr=onesrv,
                            eye=eyev))
    return in_maps


_NC_CACHE = {}


def kernel(hidden_states, cos, sin, Wq, Wk, Wv, Wo, _trace=False):
    hidden_states = np.asarray(hidden_states)
    cos = np.asarray(cos)
    sin = np.asarray(sin)
    Wq, Wk, Wv, Wo = (np.asarray(a) for a in (Wq, Wk, Wv, Wo))
    B, S, HID = hidden_states.shape
    QCH = 512

    key = (B, S, HID)
    nc = _NC_CACHE.get(key)
    if nc is None:
        nc = _NC_CACHE[key] = build_core_kernel(B, S, HID)

    in_maps = shard_inputs(hidden_states, cos, sin, Wq, Wk, Wv, Wo)
    res = run_bass_kernel_spmd(nc, in_maps, core_ids=list(range(N_CORES)),
                               trace=_trace)
    kernel._last_results = res

    acc = res.results[0]["out"].astype(np.float32)
    for r in res.results[1:]:
        acc += r["out"].astype(np.float32)
    # [B, TT, HC, 128, QCH] -> [B, S, HID]
    TT = S // 128
    HC = HID // QCH
    full = acc.transpose(0, 1, 3, 2, 4).reshape(B, S, HID)
    return np.ascontiguousarray(full)


# revision 23
# speedup vs baseline: 1.2163x; 1.0027x over previous
# Trainium2 Bass kernel: Llama-style attention block (GQA + RoPE + causal),
# tensor-parallel across heads on 8 NeuronCores.
#
# Full-shape contract: kernel(**inputs) takes the unsharded numpy inputs and
# returns the full [B, S, HID] float32 output.
#
# Sharding strategy (per core i of 8):
#   - 4 query heads (rows i*512:(i+1)*512 of Wq) + 1 kv head (rows i*128.. of Wk/Wv)
#   - Wo is sharded row-wise (its columns i*512:(i+1)*512); each core emits a
#     partial [B,S,HID] product (bf16) which the host sums at gather time.
# All weights/activations are pre-transposed and pre-tiled on the host so the
# device kernel needs zero on-chip transposes of activations:
#   matmul(out[M,N], lhsT[K,M], rhs[K,N]) contracts over the partition dim K.
#
# Schedule: fully chunk-interleaved so the PE never idles long enough for the
# HAM clock gate to re-throttle, and so the scalar engine's exp work (which
# alone nearly matches the PE's score+AV matmul time) always has PE work to
# hide behind:
#   per 512-token chunk: QKV projection sweep -> RoPE -> V transpose ->
#   attention for the chunk's 4 heads, with (a) score matmuls software-
#   pipelined two steps ahead of the attn*V matmuls, (b) the softmax
#   normalization (all-bf16 sum + broadcast matmuls, fast-approx reciprocal)
#   deferred into the NEXT head's score stream, and (c) the PREVIOUS chunk's
#   output-projection groups interleaved one-per-kt into the score stream so
#   the PE stays busy when the exp stream is the rate limiter.
# Scores stay transposed [k, q]; softmax is unnormalized (scores are O(10) so
# exp is safe in fp32); the normalizer is folded in at the attention-output
# eviction.

import os
import sys
from contextlib import ExitStack

for _p in ("/opt/trn_rl_repo", "/root/.axon_site/_ro/trn_rl_repo"):
    if os.path.isdir(_p) and _p not in sys.path:
        sys.path.append(_p)

import ml_dtypes
import numpy as np

import concourse.bass as bass
import concourse.mybir as mybir
import concourse.tile as tile
from concourse import bacc
from concourse.bass_utils import run_bass_kernel_spmd

BF16 = mybir.dt.bfloat16
F32 = mybir.dt.float32
NEG = -1.0e9
N_CORES = 8


def build_core_kernel(B, S, HID, QH=4, D=128, QCH=512):
    """SPMD per-core program. QH query heads + 1 kv head per core.

    DRAM parameter layouts (host pre-tiles everything):
      ht   [B, KT, TC, 128, QCH]  bf16  hidden^T tiles: [b,kt,tc,i,j] = hidden[b, tc*QCH+j, kt*128+i]
      trig [B, 2, D, S]           bf16  cos^T / sin^T
      wq   [KT, 128, QH*D]        bf16  Wq_core^T tiles (contraction rows blocked by 128)
      wk   [KT, 128, D]           bf16
      wv   [KT, 128, D]           bf16
      wo   [QH, D, HID]           bf16  Wo_core^T rows blocked per head
      mask [128, QCH]             f32   additive causal triangle (cols 0:128), 0 beyond
      ones [128, 1]               bf16  partition-sum lhsT
      onesr [1, 128]              bf16  broadcast lhsT
      eye  [128, 128]             bf16
      out  [B, TT, HC, 128, QCH]  bf16  partial output tiles
    """
    FS = QH * D          # per-core feature slice of the qkv/attn space
    KT = HID // 128      # contraction tiles for projections
    TC = S // QCH        # 512-token chunks
    TT = S // 128        # 128-token tiles
    TPC = QCH // 128     # token tiles per chunk
    HC = HID // QCH      # output hid chunks
    KPQ = QCH // 128     # k-tiles per q-chunk (diagonal band width)
    HALF = D // 2
    PIPE = 2             # score->AV software pipeline depth
    SC = float(1.0 / np.sqrt(D))
    EXP = mybir.ActivationFunctionType.Exp

    # Bacc (not plain Bass): its compile pipeline splits multi-sem waits into
    # EventSemaphore instructions — the DMA DIRECT2D struct has one wait slot.
    nc = bacc.Bacc(None)
    ht = nc.declare_dram_parameter("ht", [B, KT, TC, 128, QCH], BF16, isOutput=False)
    trig = nc.declare_dram_parameter("trig", [B, 2, D, S], BF16, isOutput=False)
    wq = nc.declare_dram_parameter("wq", [KT, 128, FS], BF16, isOutput=False)
    wk = nc.declare_dram_parameter("wk", [KT, 128, D], BF16, isOutput=False)
    wv = nc.declare_dram_parameter("wv", [KT, 128, D], BF16, isOutput=False)
    wo = nc.declare_dram_parameter("wo", [QH, D, HID], BF16, isOutput=False)
    mask = nc.declare_dram_parameter("mask", [128, QCH], F32, isOutput=False)
    ones = nc.declare_dram_parameter("ones", [128, 1], BF16, isOutput=False)
    onesr = nc.declare_dram_parameter("onesr", [1, 128], BF16, isOutput=False)
    eye = nc.declare_dram_parameter("eye", [128, 128], BF16, isOutput=False)
    out = nc.declare_dram_parameter("out", [B, TT, HC, 128, QCH], BF16, isOutput=True)

    with ExitStack() as ctx:
        tc = ctx.enter_context(tile.TileContext(nc))
        pool = lambda name, bufs, space=None: ctx.enter_context(
            tc.tile_pool(name=name, bufs=bufs, **({"space": space} if space else {}))
        )
        p_w = pool("p_w", 1)          # weights + constants, loaded once
        p_ht = pool("p_ht", 35)       # streamed hidden^T tiles (full chunk + prefetch)
        p_qt = pool("p_qt", 8)        # per-chunk per-head Q^T [D, QCH] bf16
        p_kt = pool("p_kt", 1)        # K^T [D, S] bf16 (persistent per batch)
        p_vtt = pool("p_vtt", 3)      # V^T staging before transpose
        p_vt = pool("p_vt", S // 128 + 4)  # V tiles [128 tok, D]
        p_at = pool("p_at", 8)        # per-chunk per-head attn^T [D, QCH] bf16
        p_exp = pool("p_exp", 7)      # exp(score) tiles bf16
        p_rt = pool("p_rt", 1)        # rope temps f32
        p_acc = pool("p_acc", 2)      # softmax-sum accumulators bf16
        p_sums = pool("p_sums", 2)    # softmax sums row bf16
        p_rbc = pool("p_rbc", 2)      # broadcast reciprocal f32
        p_ost = pool("p_ost", 4)      # output staging bf16
        p_pp = pool("p_pp", 4, "PSUM")    # general [128, QCH] psum ring
        p_av = pool("p_av", 3, "PSUM")    # attn-v accumulators (+proj V)
        p_sm = pool("p_sm", 1, "PSUM")    # softmax sum rows

        wq_sb = p_w.tile([128, KT, FS], BF16, name="wq_sb")
        wk_sb = p_w.tile([128, KT, D], BF16, name="wk_sb")
        wv_sb = p_w.tile([128, KT, D], BF16, name="wv_sb")
        wo_sb = p_w.tile([128, QH, HID], BF16, name="wo_sb")
        mask_sb = p_w.tile([128, QCH], F32, name="mask_sb")
        ones_sb = p_w.tile([128, 1], BF16, name="ones_sb")
        onesr_sb = p_w.tile([1, 128], BF16, name="onesr_sb")
        eye_sb = p_w.tile([128, 128], BF16, name="eye_sb")
        cos_sb = p_w.tile([D, B, S], BF16, name="cos_sb")
        sin_sb = p_w.tile([D, B, S], BF16, name="sin_sb")
        # Startup order: tiny constants first, then per-kt weights split
        # across the two DMA queues so the first projection sweep's operands
        # stay ahead of the PE's ~1.1 us/kt consumption rate. cos/sin are
        # needed ~35 us in (first RoPE eviction), wo at ~90 us (first outproj).
        nc.scalar.dma_start(out=mask_sb[:, :], in_=mask[:, :])
        nc.scalar.dma_start(out=ones_sb[:, :], in_=ones[:, :])
        nc.scalar.dma_start(out=onesr_sb[:, :], in_=onesr[:, :])
        nc.scalar.dma_start(out=eye_sb[:, :], in_=eye[:, :])
        for kt in range(KT):
            nc.scalar.dma_start(out=wq_sb[:, kt, :], in_=wq[kt])
            nc.sync.dma_start(out=wk_sb[:, kt, :], in_=wk[kt])
        for b in range(B):
            nc.scalar.dma_start(out=cos_sb[:, b, :], in_=trig[b, 0])
            nc.scalar.dma_start(out=sin_sb[:, b, :], in_=trig[b, 1])
        for kt in range(KT):
            nc.scalar.dma_start(out=wv_sb[:, kt, :], in_=wv[kt])
        for f in range(QH):
            nc.scalar.dma_start(out=wo_sb[:, f, :], in_=wo[f])

        def rope_evict(b, dst, ps, tci):
            # dst[:, :] = ps * cos + rotate_half(ps) * sin  (write bf16, chunk tci)
            # bf16 temporaries: DVE runs 2 elems/cycle for 16-bit vs 1 for f32.
            sl = slice(tci * QCH, (tci + 1) * QCH)
            cs = cos_sb[:, b, sl]
            sn = sin_sb[:, b, sl]
            t1 = p_rt.tile([128, QCH], BF16, name="rt1", tag="rt1")
            t2 = p_rt.tile([128, QCH], BF16, name="rt2", tag="rt2")
            nc.vector.tensor_mul(t1[:, :], ps[:, :], cs)
            nc.vector.tensor_mul(t2[0:HALF, :], ps[HALF:D, :], sn[0:HALF, :])
            nc.vector.tensor_mul(t2[HALF:D, :], ps[0:HALF, :], sn[HALF:D, :])
            nc.vector.tensor_sub(dst[0:HALF, :], t1[0:HALF, :], t2[0:HALF, :])
            nc.vector.tensor_add(dst[HALF:D, :], t1[HALF:D, :], t2[HALF:D, :])

        # Deferred softmax normalization: `pend` carries (acc, av, at_dst) of
        # the previous head; its PE ops (sum + broadcast matmuls, all bf16)
        # are issued inside the NEXT head's score stream so their
        # vector/scalar dependencies are long satisfied.
        def emit_norm_sum(pend):
            acc, av, at_dst = pend
            sm = p_sm.tile([1, QCH], F32, name="ps_sm", tag="sm")
            nc.tensor.matmul(sm[:, :], ones_sb[:, :], acc[:, :],
                             start=True, stop=True)
            sms = p_sums.tile([1, QCH], BF16, name="sms", tag="sums")
            nc.scalar.copy(sms[:, :], sm[:, :])
            return (av, at_dst, sms)

        def emit_norm_apply(pend2):
            av, at_dst, sms = pend2
            # broadcast sums down partitions via K=1 outer product
            pb = p_pp.tile([128, QCH], F32, name="ps_pb", tag="pp")
            nc.tensor.matmul(pb[:, :], onesr_sb[:, :], sms[:, :],
                             start=True, stop=True)
            rb = p_rbc.tile([128, QCH], F32, name="rb", tag="rb")
            nc.vector.reciprocal_approx_fast(rb[:, :], pb[:, :])
            nc.vector.tensor_mul(at_dst[:, :], av[:, :], rb[:, :])

        # Output projection emitters for one chunk (32 (tt,hc) groups); popped
        # one-per-kt into the next chunk's attention score stream.
        def emit_op_group(g):
            bb, tt, tt4, hc, atp = g
            csl = slice(tt4 * 128, (tt4 + 1) * 128)
            pw = p_pp.tile([128, QCH], F32, name="ps_wo", tag="pp")
            for f in range(QH):
                nc.tensor.matmul(pw[:, :], atp[f][:, csl],
                                 wo_sb[:, f, hc * QCH:(hc + 1) * QCH],
                                 start=(f == 0), stop=(f == QH - 1))
            o = p_ost.tile([128, QCH], BF16, name="o_t", tag="ost")
            if hc % 2 == 0:
                nc.vector.tensor_copy(o[:, :], pw[:, :])
            else:
                nc.scalar.copy(o[:, :], pw[:, :])
            nc.sync.dma_start(out=out[bb, tt, hc], in_=o[:, :])

        def make_op_groups(bb, tcip, atp):
            return [(bb, tcip * TPC + tt4, tt4, hc, atp)
                    for tt4 in range(TPC) for hc in range(HC)]

        pend = None       # deferred normalization state (crosses chunks)
        pend2 = None
        prev_op = None    # (b, tci, at_c) of the chunk awaiting outproj

        for b in range(B):
            vb = []       # V tiles [128 tok, D], global k-tile index
            ktb = p_kt.tile([D, S], BF16, name="ktb", tag="kt")
            for tci in range(TC):
                # ---- projection sweep 1: Q heads + K accumulate per kt ----
                qps = [p_pp.tile([128, QCH], F32, name=f"ps_q{f}", tag="pp")
                       for f in range(QH)]
                kps = p_sm.tile([128, QCH], F32, name="ps_k", tag="sm")
                hts = []
                for kt in range(KT):
                    t = p_ht.tile([128, QCH], BF16, name="ht_t", tag="ht")
                    nc.sync.dma_start(out=t[:, :], in_=ht[b, kt, tci])
                    hts.append(t)
                    st_, sp_ = (kt == 0), (kt == KT - 1)
                    for f in range(QH):
                        nc.tensor.matmul(qps[f][:, :],
                                         wq_sb[:, kt, f * D:(f + 1) * D],
                                         t[:, :], start=st_, stop=sp_)
                    nc.tensor.matmul(kps[:, :], wk_sb[:, kt, :], t[:, :],
                                     start=st_, stop=sp_)
                qtb_c = [p_qt.tile([D, QCH], BF16, name=f"qtc{f}", tag="qt")
                         for f in range(QH)]
                ktb_c = ktb[:, tci * QCH:(tci + 1) * QCH]
                # RoPE order on the DVE: K first (the score matmuls' stationary
                # operand), then q0/q1; vtt is copied mid-stream so the PE's
                # transposes unblock right after sweep 2; q2/q3 follow (their
                # heads start much later).
                rope_evict(b, ktb_c, kps, tci)
                for f in range(QH):
                    rope_evict(b, qtb_c[f], qps[f], tci)
                # ---- sweep 2: V^T accumulation, then PE-transpose ----
                vps = p_av.tile([128, QCH], F32, name="ps_v", tag="av")
                for kt in range(KT):
                    nc.tensor.matmul(vps[:, :], wv_sb[:, kt, :], hts[kt][:, :],
                                     start=(kt == 0), stop=(kt == KT - 1))
                vtt = p_vtt.tile([128, QCH], BF16, name="vtt", tag="vtt")
                nc.vector.tensor_copy(vtt[:, :], vps[:, :])
                for sub in range(TPC):
                    pt = p_pp.tile([128, 128], BF16, name="ps_vt", tag="pp")
                    nc.tensor.transpose(pt[:, :], vtt[:, sub * 128:(sub + 1) * 128],
                                        eye_sb[:, :])
                    v = p_vt.tile([128, D], BF16, name="v_t", tag="vt")
                    nc.scalar.copy(v[:, :], pt[:, :])
                    vb.append(v)

                # ---- attention for this chunk (scores transposed [k, q]),
                #      previous chunk's outproj groups interleaved ----
                oq = make_op_groups(*prev_op) if prev_op else []
                at_c = [p_at.tile([D, QCH], BF16, name=f"atc{f}", tag="at")
                        for f in range(QH)]
                nk = KPQ * (tci + 1)
                for h in range(QH):
                    av = p_av.tile([128, QCH], F32, name="ps_av", tag="av")
                    acc = p_acc.tile([128, QCH], BF16, name="acc", tag="acc")
                    es = [None] * nk

                    def emit_av(kt):
                        e, lo, n = es[kt]
                        nc.tensor.matmul(av[:, lo:QCH], vb[kt][:, :], e[:, 0:n],
                                         start=(kt == 0), stop=(kt == nk - 1))

                    for kt in range(nk):
                        j = kt - KPQ * tci          # >=0 inside diagonal band
                        lo = max(j, 0) * 128        # first live column
                        n = QCH - lo
                        st = p_pp.tile([128, QCH], F32, name="ps_st", tag="pp")
                        nc.tensor.matmul(
                            st[:, 0:n], ktb[:, kt * 128:(kt + 1) * 128],
                            qtb_c[h][:, lo:QCH], start=True, stop=True)
                        if j >= 0:  # diagonal tile: triangular mask on 1st 128
                            nc.vector.tensor_add(st[:, 0:128], st[:, 0:128],
                                                 mask_sb[:, 0:128])
                        e = p_exp.tile([128, QCH], BF16, name="e_t", tag="exp")
                        nc.scalar.activation(e[:, 0:n], st[:, 0:n], EXP, scale=SC)
                        if kt == 0:
                            nc.vector.tensor_copy(acc[:, :], e[:, :])
                        else:
                            nc.vector.tensor_add(acc[:, lo:QCH], acc[:, lo:QCH],
                                                 e[:, 0:n])
                        es[kt] = (e, lo, n)
                        if kt == 1 and pend is not None:
                            pend2 = emit_norm_sum(pend)
                            pend = None
                        if kt == 2 and pend2 is not None:
                            emit_norm_apply(pend2)
                            pend2 = None
                        if kt >= PIPE:
                            emit_av(kt - PIPE)
                        if kt >= 3 and oq:
                            emit_op_group(oq.pop(0))
                    for kt in range(max(0, nk - PIPE), nk):
                        emit_av(kt)
                    pend = (acc, av, at_c[h])
                # leftover outproj groups of the previous chunk
                for g in oq:
                    emit_op_group(g)
                prev_op = (b, tci, at_c)

        # ---- tail: flush the last head's normalization + final outproj ----
        pend2 = emit_norm_sum(pend)
        pend = None
        emit_norm_apply(pend2)
        pend2 = None
        for g in make_op_groups(*prev_op):
            emit_op_group(g)
    nc.finalize()  # Bacc: runs compile() (reg alloc, wait splitting) + freeze
    return nc


def shard_inputs(hidden_states, cos, sin, Wq, Wk, Wv, Wo, n_cores=N_CORES,
                 QH=4, D=128, QCH=512):
    """Host-side prep: transpose/tile/bf16-round everything per core."""
    bf16 = ml_dtypes.bfloat16
    B, S, HID = hidden_states.shape
    FS = QH * D
    KT = HID // 128
    TC = S // QCH

    hT = hidden_states.astype(bf16).transpose(0, 2, 1)           # [B, HID, S]
    ht_t = np.ascontiguousarray(
        hT.reshape(B, KT, 128, TC, QCH).transpose(0, 1, 3, 2, 4))
    trig = np.ascontiguousarray(np.stack(
        [cos.transpose(0, 2, 1), sin.transpose(0, 2, 1)], axis=1)
    ).astype(bf16)

    kk = np.arange(128)[:, None]
    cc = np.arange(QCH)[None, :]
    maskv = np.where(cc < kk, np.float32(NEG), np.float32(0.0))
    onesv = np.ones((128, 1), dtype=bf16)
    onesrv = np.ones((1, 128), dtype=bf16)
    eyev = np.eye(128, dtype=bf16)

    in_maps = []
    for i in range(n_cores):
        wq_i = Wq[i * FS:(i + 1) * FS, :].T.astype(bf16).reshape(KT, 128, FS)
        wk_i = Wk[i * D:(i + 1) * D, :].T.astype(bf16).reshape(KT, 128, D)
        wv_i = Wv[i * D:(i + 1) * D, :].T.astype(bf16).reshape(KT, 128, D)
        wo_i = Wo[:, i * FS:(i + 1) * FS].T.astype(bf16).reshape(QH, D, HID)
        in_maps.append(dict(ht=ht_t, trig=trig, wq=wq_i, wk=wk_i, wv=wv_i,
                            wo=wo_i, mask=maskv, ones=onesv, onesr=onesrv,
                            eye=eyev))
    return in_maps


_NC_CACHE = {}


def kernel(hidden_states, cos, sin, Wq, Wk, Wv, Wo, _trace=False):
    hidden_states = np.asarray(hidden_states)
    cos = np.asarray(cos)
    sin = np.asarray(sin)
    Wq, Wk, Wv, Wo = (np.asarray(a) for a in (Wq, Wk, Wv, Wo))
    B, S, HID = hidden_states.shape
    QCH = 512

    key = (B, S, HID)
    nc = _NC_CACHE.get(key)
    if nc is None:
        nc = _NC_CACHE[key] = build_core_kernel(B, S, HID)

    in_maps = shard_inputs(hidden_states, cos, sin, Wq, Wk, Wv, Wo)
    res = run_bass_kernel_spmd(nc, in_maps, core_ids=list(range(N_CORES)),
                               trace=_trace)
    kernel._last_results = res

    acc = res.results[0]["out"].astype(np.float32)
    for r in res.results[1:]:
        acc += r["out"].astype(np.float32)
    # [B, TT, HC, 128, QCH] -> [B, S, HID]
    TT = S // 128
    HC = HID // QCH
    full = acc.transpose(0, 1, 3, 2, 4).reshape(B, S, HID)
    return np.ascontiguousarray(full)
